# revision 22
# baseline (speedup 1.0000x reference)
"""Trainium2 Bass kernel for Disk descriptor mutual-NN matching (retrieval_knn).

Strategy (8 NeuronCores, shard descriptors1 columns M across cores):
  Each core computes S = (8*d0)^T (8*d1_shard) via fp8 DoubleRow matmuls
  (64 chunks of [128, 1024] PSUM fp32), reduces row-sibling chunk pairs
  with an elementwise max into a compact fp8 candidate map; all exact
  arithmetic happens on the host over tiny candidate sets.

  PSUM->SBUF evacuation is the hard bottleneck: on TRN2 only ACT and DVE
  may read PSUM (GPSIMD cannot access PSUM nor run TensorTensor at all;
  two PSUM inputs on one instruction are illegal - NCC_IBVF027; DMA
  cannot source PSUM), both at 1 elem/cycle/partition, so the 64 chunk
  reads floor at ~36.4us across the two lanes. Each element is read
  exactly once:
    - 29 fused pairs: ACT copies chunk 2j -> SBUF f16 (1038ns); DVE
      fuses the second PSUM read into max(PSUM_{2j+1}, sp) -> fp8
      (1192ns).
    - 3 TYPE_B pairs (5, 16, 30): BOTH chunks copied by ACT and the
      pair-max runs all-f16 on DVE in its 2x mode (594ns) -> f16 out
      (fp8 out would break the 2x mode), rebalancing ACT 35x1038=36.3us
      vs DVE 29x1192 + 3x594 + split tail = 36.5us. The second TYPE_B
      copy is deferred past the next pair's first copy so DVE's fused
      max is not starved.
  The last pair's max is split in [512] halves so the first half's
  flush DMA overlaps the second half (shorter tail).

  u_j[p, c] = max(S[256j+p, c], S[256j+128+p, c]) serves BOTH
  directions on host (fp8e4m3 for fused pairs, f16 for TYPE_B):
    - forward:  row r's scores are map (r//256, r%128) -> host top-64
      columns cover the true top-2 (fp8 quantization needs deeper
      candidates than f16; measured 0 misses at 48).
    - backward: column j's block scores over 4096 2-row blocks -> host
      top-16 blocks (32 rows; measured 0 misses at 12).
  Host computes exact fp32 dots for the candidates only and applies the
  reference's exact ratio-test / mutual-NN arithmetic.
"""

import sys

if "/opt/trn_rl_repo" not in sys.path:
    sys.path.insert(0, "/opt/trn_rl_repo")

import numpy as np
import ml_dtypes

N_KPTS = 8192
M_KPTS = 8192
F_DIM = 256
N_CORES = 8
M_SHARD = M_KPTS // N_CORES      # 1024

N_CHUNKS = N_KPTS // 128         # 64 row chunks
N_PAIRS = N_CHUNKS // 2          # 32 chunk pairs (2-row blocks)

RBWD_W = N_PAIRS * M_SHARD       # 32768

FP8_SCALE = np.float32(8.0)

SQRT_2 = np.float32(1.414213)
CLIP_LO = np.float32(1e-6)
ONE = np.float32(1.0)

TOPC_FWD = 64                    # candidate columns per row
TOPB_BWD = 16                    # 2-row blocks per column (32 rows)

# pairs where BOTH conversions run on ACT and the pair-max runs f16 on
# DVE at 2x rate (594ns); the other 29 pairs: ACT copies chunk A, DVE
# fuses the second PSUM read into max(PSUM_B, spA). Balance: ACT
# 35x1038=36.3us, DVE 29x1192 + 3x594 = 36.4us. (GPSIMD cannot touch
# PSUM nor run TensorTensor on TRN2, so only 2 lanes exist.)
TYPE_B_PAIRS = frozenset((5, 16, 30))
N_TYPE_B = len(TYPE_B_PAIRS)
DEFER_TYPE_B = True


def build_kernel():
    import concourse.bacc as bacc
    import concourse.mybir as mybir
    import concourse.tile as tile

    nc = bacc.Bacc("TRN2", target_bir_lowering=False, debug=False,
                   num_devices=1)

    d0dr = nc.dram_tensor("d0dr", [128, 2, N_KPTS], mybir.dt.float8e4,
                          kind="ExternalInput")
    d1dr = nc.dram_tensor("d1dr", [128, 2, M_SHARD], mybir.dt.float8e4,
                          kind="ExternalInput")
    n_b = len(TYPE_B_PAIRS)
    rbwd = nc.dram_tensor("rbwd", [128, (N_PAIRS - n_b) * M_SHARD],
                          mybir.dt.float8e4, kind="ExternalOutput")
    rbwd16 = nc.dram_tensor("rbwd16", [128, n_b * M_SHARD],
                            mybir.dt.float16, kind="ExternalOutput")

    mx = mybir.AluOpType.max
    DR = mybir.MatmulPerfMode.DoubleRow

    with tile.TileContext(nc) as tc:
        with tc.tile_pool(name="persist", bufs=1) as persist, \
             tc.tile_pool(name="s16", bufs=6) as s16_pool, \
             tc.tile_pool(name="outs", bufs=1) as outs_pool, \
             tc.tile_pool(name="psf", bufs=4, space="PSUM") as psf:

            d0s = persist.tile([128, 2, N_KPTS], mybir.dt.float8e4,
                               name="d0s")
            d1s = persist.tile([128, 2, M_SHARD], mybir.dt.float8e4,
                               name="d1s")
            # tiny first pieces so the first matmul can start immediately;
            # d1 rides the gpsimd SWDGE queue so its latency overlaps SP's
            nc.sync.dma_start(d0s[:, :, 0:128], d0dr[:, :, 0:128])
            nc.gpsimd.dma_start(d1s[:, :, 0:512], d1dr[:, :, 0:512])
            nc.gpsimd.dma_start(d1s[:, :, 512:1024], d1dr[:, :, 512:1024])
            bounds = [128, 2048, 4096, 6144, 8192]
            for p in range(len(bounds) - 1):
                sl = slice(bounds[p], bounds[p + 1])
                nc.sync.dma_start(d0s[:, :, sl], d0dr[:, :, sl])

            u_out = outs_pool.tile([128, N_PAIRS - n_b, M_SHARD],
                                   mybir.dt.float8e4, name="u_out")
            u16_out = outs_pool.tile([128, max(1, n_b), M_SHARD],
                                     mybir.dt.float16, name="u16_out")



            def chunk_matmuls(n):
                pf = psf.tile([128, M_SHARD], mybir.dt.float32, tag="pf")
                for m in range(2):
                    nc.tensor.matmul(
                        pf[:, m * 512:(m + 1) * 512],
                        d0s[:, :, n * 128:(n + 1) * 128],
                        d1s[:, :, m * 512:(m + 1) * 512],
                        start=True, stop=True, perf_mode=DR)
                return pf

            s8 = 0      # next fused-pair slot in u_out
            s16 = 0     # next TYPE_B slot in u16_out
            flush_lo = 0
            pend16 = None   # deferred TYPE_B combine (sp0, sp1, slot)
            for j in range(N_PAIRS):
                pf0 = chunk_matmuls(2 * j)
                sp0 = s16_pool.tile([128, M_SHARD], mybir.dt.float16,
                                    tag="sp")
                nc.scalar.copy(sp0[:], pf0[:])
                if pend16 is not None:
                    # emit the previous TYPE_B pair's second copy AFTER the
                    # next pair's sp0 so DVE's fused max is never starved
                    # waiting on ACT (was a 344ns DVE gap per TYPE_B pair)
                    q0, q1p, slot = pend16
                    nc.scalar.copy(q1p[0][:], q1p[1][:])
                    # f16 in/out keeps DVE in 2x mode (594ns vs 1192)
                    nc.vector.tensor_tensor(out=u16_out[:, slot, :],
                                            in0=q0[:], in1=q1p[0][:], op=mx)
                    nc.sync.dma_start(
                        rbwd16[:, slot * M_SHARD:(slot + 1) * M_SHARD],
                        u16_out[:, slot, :])
                    pend16 = None
                pf1 = chunk_matmuls(2 * j + 1)
                if j in TYPE_B_PAIRS:
                    sp1 = s16_pool.tile([128, M_SHARD], mybir.dt.float16,
                                        tag="sp")
                    if DEFER_TYPE_B:
                        pend16 = (sp0, (sp1, pf1), s16)
                    else:
                        nc.scalar.copy(sp1[:], pf1[:])
                        nc.vector.tensor_tensor(out=u16_out[:, s16, :],
                                                in0=sp0[:], in1=sp1[:],
                                                op=mx)
                        nc.sync.dma_start(
                            rbwd16[:, s16 * M_SHARD:(s16 + 1) * M_SHARD],
                            u16_out[:, s16, :])
                    s16 += 1
                    continue
                if j == N_PAIRS - 1:
                    # split the last fused op so its first half's DMA
                    # overlaps the second half (shorter tail)
                    for hh in range(2):
                        sl2 = slice(hh * 512, (hh + 1) * 512)
                        nc.vector.tensor_tensor(out=u_out[:, s8, sl2],
                                                in0=pf1[:, sl2],
                                                in1=sp0[:, sl2], op=mx)
                        nc.sync.dma_start(
                            rbwd[:, s8 * M_SHARD + hh * 512:
                                 s8 * M_SHARD + (hh + 1) * 512],
                            u_out[:, s8, sl2])
                    s8 += 1
                    continue
                # fused: second PSUM read + pair-max in one DVE op
                nc.vector.tensor_tensor(out=u_out[:, s8, :], in0=pf1[:],
                                        in1=sp0[:], op=mx)
                s8 += 1
                # stream u out in batches; finer near the end for the tail
                if (s8 - flush_lo >= 4) or (j >= 28 and s8 > flush_lo):
                    sl = slice(flush_lo * M_SHARD, s8 * M_SHARD)
                    nc.sync.dma_start(
                        rbwd[:, sl],
                        u_out[:, flush_lo:s8, :].rearrange(
                            "p a b -> p (a b)"))
                    flush_lo = s8

    nc.compile()
    return nc


_KERNEL_CACHE = {}


def get_kernel():
    if "k" not in _KERNEL_CACHE:
        _KERNEL_CACHE["k"] = build_kernel()
    return _KERNEL_CACHE["k"]


# --------------------------------------------------------------------------
# Host side
# --------------------------------------------------------------------------

def make_core_inputs(d0, d1):
    """d0, d1: [256, 8192] float32 (full). Returns per-core input dicts."""
    d0_8 = (d0 * FP8_SCALE).astype(ml_dtypes.float8_e4m3fn)
    d1_8 = (d1 * FP8_SCALE).astype(ml_dtypes.float8_e4m3fn)
    # DoubleRow layout: [k, t, i] = x[t*128 + k, i]
    d0dr = np.ascontiguousarray(
        d0_8.reshape(2, 128, N_KPTS).transpose(1, 0, 2))
    in_maps = []
    for c in range(N_CORES):
        sh = d1_8[:, c * M_SHARD:(c + 1) * M_SHARD]
        d1dr = np.ascontiguousarray(
            sh.reshape(2, 128, M_SHARD).transpose(1, 0, 2))
        in_maps.append({"d0dr": d0dr, "d1dr": d1dr})
    return in_maps


def run_device(d0, d1):
    from concourse.bass_utils import run_bass_kernel_spmd

    nc = get_kernel()
    in_maps = make_core_inputs(d0, d1)
    last_err = None
    for _attempt in range(3):
        try:
            res = run_bass_kernel_spmd(nc, in_maps, list(range(N_CORES)))
            return res.results
        except Exception as e:  # rare transient device flakes
            last_err = e
    raise last_err


def _topk_idx(arr, k):
    """Indices of the k largest per row (unordered); torch is ~10x faster
    than np.argpartition on this host."""
    try:
        import torch
        return torch.topk(torch.from_numpy(arr), k, dim=1).indices.numpy()
    except ImportError:
        return np.argpartition(-arr, k - 1, axis=1)[:, :k]


def postprocess(results, d0, d1):
    """results: per-core {'rbwd'}; d0,d1 [256,8192] f32 full."""
    d0T = np.ascontiguousarray(d0.T)   # [N, F] f32
    d1T = np.ascontiguousarray(d1.T)   # [M, F] f32

    # u map: u[core, p, j, c] = max(S[256j+p, core*1024+c],
    #                               S[256j+128+p, core*1024+c])
    # fused pairs arrive fp8 in "rbwd" (slot order = pairs not in
    # TYPE_B_PAIRS), TYPE_B pairs arrive f16 in "rbwd16".
    rb8 = np.stack([np.asarray(r["rbwd"]).view(ml_dtypes.float8_e4m3fn)
                    for r in results]).astype(np.float32)
    rb8 = rb8.reshape(N_CORES, 128, N_PAIRS - N_TYPE_B, M_SHARD)
    rb16 = np.stack([np.asarray(r["rbwd16"]) for r in results]
                    ).astype(np.float32)
    rb16 = rb16.reshape(N_CORES, 128, N_TYPE_B, M_SHARD)
    rb = np.empty((N_CORES, 128, N_PAIRS, M_SHARD), np.float32)
    tb = sorted(TYPE_B_PAIRS)
    fused = [j for j in range(N_PAIRS) if j not in TYPE_B_PAIRS]
    rb[:, :, fused, :] = rb8
    rb[:, :, tb, :] = rb16
    # bm[(j, p), global col] -- one map per 2-row block
    bm = np.ascontiguousarray(
        rb.transpose(2, 1, 0, 3).reshape(N_PAIRS * 128, M_KPTS))

    # ---- forward: rows r and r^128 share map (r//256, r%128) ----
    topc = _topk_idx(bm, TOPC_FWD)                       # [4096, K]
    r_all = np.arange(N_KPTS)
    map_id = (r_all // 256) * 128 + (r_all % 128)
    js = topc[map_id]                                   # [N, K] candidate cols

    s1 = np.empty(N_KPTS, np.float32)
    s2 = np.empty(N_KPTS, np.float32)
    fwd_nn = np.empty(N_KPTS, np.int64)
    slab = 2048
    for s in range(0, N_KPTS, slab):
        e = s + slab
        gath = d1T[js[s:e]]                                # [slab, K, F]
        dots = (gath * d0T[s:e, None, :]).sum(-1)          # [slab, K] f32
        m1 = dots.max(axis=1)
        nn = np.where(dots == m1[:, None], js[s:e], M_KPTS + 1).min(axis=1)
        mk = np.where(js[s:e] == nn[:, None], -np.inf, dots)
        s1[s:e] = m1
        s2[s:e] = mk.max(axis=1)
        fwd_nn[s:e] = nn

    # ---- backward: per column, top blocks over the 4096 2-row blocks ----
    bmT = np.ascontiguousarray(bm.T)                       # [M, 4096]
    topb = _topk_idx(bmT, TOPB_BWD)
    jj, pp = np.divmod(topb, 128)
    rows = np.stack([jj * 256 + pp, jj * 256 + 128 + pp],
                    axis=2).reshape(M_KPTS, -1)            # [M, 2*TOPB]

    cm1 = np.empty(M_KPTS, np.float32)
    cm2 = np.empty(M_KPTS, np.float32)
    bck_nn = np.empty(M_KPTS, np.int64)
    for s in range(0, M_KPTS, slab):
        e = s + slab
        g2 = d0T[rows[s:e]]                          # [slab, 2*TOPB, F]
        dd = (g2 * d1T[s:e, None, :]).sum(-1)        # [slab, 2*TOPB] f32
        m1 = dd.max(axis=1)
        nn = np.where(dd == m1[:, None], rows[s:e], N_KPTS + 1).min(axis=1)
        mk = np.where(rows[s:e] == nn[:, None], -np.inf, dd)
        cm1[s:e] = m1
        cm2[s:e] = mk.max(axis=1)
        bck_nn[s:e] = nn

    # ---- exact reference arithmetic (float32) ----
    def dist(s):
        return SQRT_2 * np.sqrt(np.maximum(ONE - s.astype(np.float32),
                                           CLIP_LO))

    fwd_ok = (dist(s1) / dist(s2)) < ONE
    bck_ok = (dist(cm1) / dist(cm2)) < ONE

    mutual = fwd_ok & bck_ok[fwd_nn] & (bck_nn[fwd_nn] == np.arange(N_KPTS))

    indices0 = np.where(mutual, fwd_nn, -1)[None, :].astype(np.int32)
    mscores0 = (indices0 > 0).astype(np.int32)
    matches1 = np.full((1, M_KPTS), -1, dtype=np.int32)
    mscores1 = np.zeros((1, M_KPTS), dtype=np.float32)
    return indices0, matches1, mscores0, mscores1


def kernel(descriptors0, descriptors1, keypoints0, keypoints1):
    d0 = np.ascontiguousarray(descriptors0[0]).astype(np.float32, copy=False)
    d1 = np.ascontiguousarray(descriptors1[0]).astype(np.float32, copy=False)
    results = run_device(d0, d1)
    return postprocess(results, d0, d1)


# revision 36
# speedup vs baseline: 1.0168x; 1.0168x over previous
"""Trainium2 Bass kernel for Disk descriptor mutual-NN matching (retrieval_knn).

Strategy (8 NeuronCores, shard descriptors1 columns M across cores):
  Each core computes S = (8*d0)^T (8*d1_shard) via fp8 DoubleRow matmuls
  (64 chunks of [128, 1024] PSUM fp32), reduces row-sibling chunk pairs
  with an elementwise max into a compact fp8 candidate map; all exact
  arithmetic happens on the host over tiny candidate sets.

  PSUM->SBUF evacuation is the hard bottleneck: on TRN2 only ACT and DVE
  may read PSUM (GPSIMD cannot access PSUM nor run TensorTensor at all;
  two PSUM inputs on one instruction are illegal - NCC_IBVF027; DMA
  cannot source PSUM), both at 1 elem/cycle/partition, so the 64 chunk
  reads floor at ~36.4us across the two lanes. Each element is read
  exactly once:
    - 29 fused pairs: ACT copies chunk 2j -> SBUF f16 (1038ns); DVE
      fuses the second PSUM read into max(PSUM_{2j+1}, sp) -> fp8
      (1192ns).
    - 3 TYPE_B pairs (5, 16, 30): BOTH chunks copied by ACT and the
      pair-max runs all-f16 on DVE in its 2x mode (594ns) -> f16 out
      (fp8 out would break the 2x mode), rebalancing ACT 35x1038=36.3us
      vs DVE 29x1192 + 3x594 + split tail = 36.5us. The second TYPE_B
      copy is deferred past the next pair's first copy so DVE's fused
      max is not starved.
  The last pair's max is split in [512] halves so the first half's
  flush DMA overlaps the second half (shorter tail).

  u_j[p, c] = max(S[256j+p, c], S[256j+128+p, c]) serves BOTH
  directions on host (fp8e4m3 for fused pairs, f16 for TYPE_B):
    - forward:  row r's scores are map (r//256, r%128) -> host top-64
      columns cover the true top-2 (fp8 quantization needs deeper
      candidates than f16; measured 0 misses at 48).
    - backward: column j's block scores over 4096 2-row blocks -> host
      top-16 blocks (32 rows; measured 0 misses at 12).
  Host computes exact fp32 dots for the candidates only and applies the
  reference's exact ratio-test / mutual-NN arithmetic.
"""

import sys

if "/opt/trn_rl_repo" not in sys.path:
    sys.path.insert(0, "/opt/trn_rl_repo")

import numpy as np
import ml_dtypes

N_KPTS = 8192
M_KPTS = 8192
F_DIM = 256
N_CORES = 8
M_SHARD = M_KPTS // N_CORES      # 1024

N_CHUNKS = N_KPTS // 128         # 64 row chunks
N_PAIRS = N_CHUNKS // 2          # 32 chunk pairs (2-row blocks)

RBWD_W = N_PAIRS * M_SHARD       # 32768

FP8_SCALE = np.float32(8.0)

SQRT_2 = np.float32(1.414213)
CLIP_LO = np.float32(1e-6)
ONE = np.float32(1.0)

TOPC_FWD = 64                    # candidate columns per row
TOPB_BWD = 16                    # 2-row blocks per column (32 rows)

# TYPE_B pairs combine a long-held EARLY chunk with a LATE chunk (the
# chunk pairing is a host-side convention, so pairs need not be
# adjacent). Both chunks are ACT-copied and the pair-max runs all-f16
# on DVE in its 2x mode (594ns vs 1192). Non-adjacent pairing means ACT
# never issues two back-to-back copies for one pair (no pipeline burst,
# no PSUM over-hold), and the kernel's final DVE op becomes a cheap
# combine instead of a PSUM max. Balance: ACT 35x1038=36.3us, DVE
# 29x1192 + 3x594 = 36.4us. (GPSIMD cannot touch PSUM nor run
# TensorTensor on TRN2, so only these 2 lanes exist.)
TYPE_B_CHUNK_PAIRS = ((10, 61), (27, 62), (44, 63))
N_TYPE_B = len(TYPE_B_CHUNK_PAIRS)


def _chunk_pairs():
    """29 fused adjacent chunk pairs + the 3 TYPE_B split pairs."""
    special = {c for p in TYPE_B_CHUNK_PAIRS for c in p}
    rem = [c for c in range(N_CHUNKS) if c not in special]
    fused = list(zip(rem[0::2], rem[1::2]))
    return fused, list(TYPE_B_CHUNK_PAIRS)


def build_kernel():
    import concourse.bacc as bacc
    import concourse.mybir as mybir
    import concourse.tile as tile

    nc = bacc.Bacc("TRN2", target_bir_lowering=False, debug=False,
                   num_devices=1)

    d0dr = nc.dram_tensor("d0dr", [128, 2, N_KPTS], mybir.dt.float8e4,
                          kind="ExternalInput")
    d1dr = nc.dram_tensor("d1dr", [128, 2, M_SHARD], mybir.dt.float8e4,
                          kind="ExternalInput")
    n_b = N_TYPE_B
    rbwd = nc.dram_tensor("rbwd", [128, (N_PAIRS - n_b) * M_SHARD],
                          mybir.dt.float8e4, kind="ExternalOutput")
    rbwd16 = nc.dram_tensor("rbwd16", [128, n_b * M_SHARD],
                            mybir.dt.float16, kind="ExternalOutput")

    mx = mybir.AluOpType.max
    DR = mybir.MatmulPerfMode.DoubleRow

    with tile.TileContext(nc) as tc:
        with tc.tile_pool(name="persist", bufs=1) as persist, \
             tc.tile_pool(name="s16", bufs=6) as s16_pool, \
             tc.tile_pool(name="outs", bufs=1) as outs_pool, \
             tc.tile_pool(name="psf", bufs=4, space="PSUM") as psf:

            d0s = persist.tile([128, 2, N_KPTS], mybir.dt.float8e4,
                               name="d0s")
            d1s = persist.tile([128, 2, M_SHARD], mybir.dt.float8e4,
                               name="d1s")
            # tiny first pieces so the first matmul can start immediately;
            # d1 rides the gpsimd SWDGE queue so its latency overlaps SP's
            nc.sync.dma_start(d0s[:, :, 0:128], d0dr[:, :, 0:128])
            nc.gpsimd.dma_start(d1s[:, :, 0:512], d1dr[:, :, 0:512])
            nc.gpsimd.dma_start(d1s[:, :, 512:1024], d1dr[:, :, 512:1024])
            bounds = [128, 2048, 4096, 6144, 8192]
            for p in range(len(bounds) - 1):
                sl = slice(bounds[p], bounds[p + 1])
                nc.sync.dma_start(d0s[:, :, sl], d0dr[:, :, sl])

            u_out = outs_pool.tile([128, N_PAIRS - n_b, M_SHARD],
                                   mybir.dt.float8e4, name="u_out")
            u16_out = outs_pool.tile([128, max(1, n_b), M_SHARD],
                                     mybir.dt.float16, name="u16_out")



            def chunk_matmuls(n):
                pf = psf.tile([128, M_SHARD], mybir.dt.float32, tag="pf")
                for m in range(2):
                    nc.tensor.matmul(
                        pf[:, m * 512:(m + 1) * 512],
                        d0s[:, :, n * 128:(n + 1) * 128],
                        d1s[:, :, m * 512:(m + 1) * 512],
                        start=True, stop=True, perf_mode=DR)
                return pf

            fused_pairs, typeb_pairs = _chunk_pairs()
            role = {}
            for slot, (a, b) in enumerate(fused_pairs):
                role[a] = ("fa", slot)
                role[b] = ("fb", slot)
            for i, (a, b) in enumerate(typeb_pairs):
                role[a] = ("xa", i)
                role[b] = ("xb", i)
            # long-lived SBUF homes for the early TYPE_B chunks
            spx = [persist.tile([128, M_SHARD], mybir.dt.float16,
                                name=f"spx{i}") for i in range(N_TYPE_B)]

            sp_for_slot = {}
            flush_lo = 0
            s8 = 0
            for c in range(N_CHUNKS):
                pf = chunk_matmuls(c)
                kind, idx = role[c]
                if kind == "fa":
                    sp = s16_pool.tile([128, M_SHARD], mybir.dt.float16,
                                       tag="sp")
                    nc.scalar.copy(sp[:], pf[:])
                    sp_for_slot[idx] = sp
                elif kind == "fb":
                    spA = sp_for_slot.pop(idx)
                    if c == N_CHUNKS - 1:
                        # split the last fused max so its first half's DMA
                        # overlaps the second half (shorter tail)
                        for hh in range(2):
                            sl2 = slice(hh * 512, (hh + 1) * 512)
                            nc.vector.tensor_tensor(out=u_out[:, idx, sl2],
                                                    in0=pf[:, sl2],
                                                    in1=spA[:, sl2], op=mx)
                            nc.sync.dma_start(
                                rbwd[:, idx * M_SHARD + hh * 512:
                                     idx * M_SHARD + (hh + 1) * 512],
                                u_out[:, idx, sl2])
                        s8 = idx + 1
                        continue
                    # fused: second PSUM read + pair-max in one DVE op
                    nc.vector.tensor_tensor(out=u_out[:, idx, :],
                                            in0=pf[:], in1=spA[:], op=mx)
                    s8 = idx + 1
                    # stream u out in 4-slot batches; the final slot
                    # flushes as soon as it completes
                    n_fused = N_PAIRS - N_TYPE_B
                    if (s8 - flush_lo >= 4) or (s8 == n_fused):
                        sl = slice(flush_lo * M_SHARD, s8 * M_SHARD)
                        nc.sync.dma_start(
                            rbwd[:, sl],
                            u_out[:, flush_lo:s8, :].rearrange(
                                "p a b -> p (a b)"))
                        flush_lo = s8
                elif kind == "xa":
                    nc.scalar.copy(spx[idx][:], pf[:])
                else:  # "xb": late TYPE_B chunk -> copy + f16 combine
                    spy = s16_pool.tile([128, M_SHARD], mybir.dt.float16,
                                        tag="sp")
                    nc.scalar.copy(spy[:], pf[:])
                    if c == N_CHUNKS - 1:
                        # split the final combine so its first half's DMA
                        # overlaps the second half; ride the otherwise-idle
                        # ACT HWDGE queue (the SP queue is draining the
                        # batched fused flushes at this point)
                        for hh in range(2):
                            sl2 = slice(hh * 512, (hh + 1) * 512)
                            nc.vector.tensor_tensor(
                                out=u16_out[:, idx, sl2],
                                in0=spx[idx][:, sl2], in1=spy[:, sl2],
                                op=mx)
                            eng = nc.sync if hh == 0 else nc.gpsimd
                            eng.dma_start(
                                rbwd16[:, idx * M_SHARD + hh * 512:
                                       idx * M_SHARD + (hh + 1) * 512],
                                u16_out[:, idx, sl2])
                    else:
                        nc.vector.tensor_tensor(out=u16_out[:, idx, :],
                                                in0=spx[idx][:],
                                                in1=spy[:], op=mx)
                        eng = nc.sync if idx % 2 == 0 else nc.gpsimd
                        eng.dma_start(
                            rbwd16[:, idx * M_SHARD:(idx + 1) * M_SHARD],
                            u16_out[:, idx, :])

    nc.compile()
    return nc


_KERNEL_CACHE = {}


def get_kernel():
    if "k" not in _KERNEL_CACHE:
        _KERNEL_CACHE["k"] = build_kernel()
    return _KERNEL_CACHE["k"]


# --------------------------------------------------------------------------
# Host side
# --------------------------------------------------------------------------

def make_core_inputs(d0, d1):
    """d0, d1: [256, 8192] float32 (full). Returns per-core input dicts."""
    d0_8 = (d0 * FP8_SCALE).astype(ml_dtypes.float8_e4m3fn)
    d1_8 = (d1 * FP8_SCALE).astype(ml_dtypes.float8_e4m3fn)
    # DoubleRow layout: [k, t, i] = x[t*128 + k, i]
    d0dr = np.ascontiguousarray(
        d0_8.reshape(2, 128, N_KPTS).transpose(1, 0, 2))
    in_maps = []
    for c in range(N_CORES):
        sh = d1_8[:, c * M_SHARD:(c + 1) * M_SHARD]
        d1dr = np.ascontiguousarray(
            sh.reshape(2, 128, M_SHARD).transpose(1, 0, 2))
        in_maps.append({"d0dr": d0dr, "d1dr": d1dr})
    return in_maps


def run_device(d0, d1):
    from concourse.bass_utils import run_bass_kernel_spmd

    nc = get_kernel()
    in_maps = make_core_inputs(d0, d1)
    last_err = None
    for _attempt in range(3):
        try:
            res = run_bass_kernel_spmd(nc, in_maps, list(range(N_CORES)))
            return res.results
        except Exception as e:  # rare transient device flakes
            last_err = e
    raise last_err


def _topk_idx(arr, k):
    """Indices of the k largest per row (unordered); torch is ~10x faster
    than np.argpartition on this host."""
    try:
        import torch
        return torch.topk(torch.from_numpy(arr), k, dim=1).indices.numpy()
    except ImportError:
        return np.argpartition(-arr, k - 1, axis=1)[:, :k]


def postprocess(results, d0, d1):
    """results: per-core {'rbwd'}; d0,d1 [256,8192] f32 full."""
    d0T = np.ascontiguousarray(d0.T)   # [N, F] f32
    d1T = np.ascontiguousarray(d1.T)   # [M, F] f32

    # u map: slot s pairs chunks (cA, cB) of the pair table;
    # u[core, p, s, c] = max(S[cA*128+p, core*1024+c],
    #                        S[cB*128+p, core*1024+c])
    # fused pairs arrive fp8 in "rbwd" (slots 0..28), TYPE_B pairs f16 in
    # "rbwd16" (slots 29..31).
    rb8 = np.stack([np.asarray(r["rbwd"]).view(ml_dtypes.float8_e4m3fn)
                    for r in results]).astype(np.float32)
    rb8 = rb8.reshape(N_CORES, 128, N_PAIRS - N_TYPE_B, M_SHARD)
    rb16 = np.stack([np.asarray(r["rbwd16"]) for r in results]
                    ).astype(np.float32)
    rb16 = rb16.reshape(N_CORES, 128, N_TYPE_B, M_SHARD)
    rb = np.concatenate([rb8, rb16], axis=2)
    # bm[(slot, p), global col] -- one map per 2-row block
    bm = np.ascontiguousarray(
        rb.transpose(2, 1, 0, 3).reshape(N_PAIRS * 128, M_KPTS))

    fused_pairs, typeb_pairs = _chunk_pairs()
    pairs_all = fused_pairs + typeb_pairs
    chunk_a = np.array([p[0] for p in pairs_all])
    chunk_b = np.array([p[1] for p in pairs_all])
    slot_of_chunk = np.empty(N_CHUNKS, np.int64)
    for sl, (a, b) in enumerate(pairs_all):
        slot_of_chunk[a] = sl
        slot_of_chunk[b] = sl

    # ---- forward: row r's map is its chunk's slot ----
    topc = _topk_idx(bm, TOPC_FWD)                       # [4096, K]
    r_all = np.arange(N_KPTS)
    map_id = slot_of_chunk[r_all // 128] * 128 + (r_all % 128)
    js = topc[map_id]                                   # [N, K] candidate cols

    s1 = np.empty(N_KPTS, np.float32)
    s2 = np.empty(N_KPTS, np.float32)
    fwd_nn = np.empty(N_KPTS, np.int64)
    slab = 2048
    for s in range(0, N_KPTS, slab):
        e = s + slab
        gath = d1T[js[s:e]]                                # [slab, K, F]
        dots = (gath * d0T[s:e, None, :]).sum(-1)          # [slab, K] f32
        m1 = dots.max(axis=1)
        nn = np.where(dots == m1[:, None], js[s:e], M_KPTS + 1).min(axis=1)
        mk = np.where(js[s:e] == nn[:, None], -np.inf, dots)
        s1[s:e] = m1
        s2[s:e] = mk.max(axis=1)
        fwd_nn[s:e] = nn

    # ---- backward: per column, top blocks over the 4096 2-row blocks ----
    bmT = np.ascontiguousarray(bm.T)                       # [M, 4096]
    topb = _topk_idx(bmT, TOPB_BWD)
    jj, pp = np.divmod(topb, 128)                          # jj = slot
    rows = np.stack([chunk_a[jj] * 128 + pp, chunk_b[jj] * 128 + pp],
                    axis=2).reshape(M_KPTS, -1)            # [M, 2*TOPB]

    cm1 = np.empty(M_KPTS, np.float32)
    cm2 = np.empty(M_KPTS, np.float32)
    bck_nn = np.empty(M_KPTS, np.int64)
    for s in range(0, M_KPTS, slab):
        e = s + slab
        g2 = d0T[rows[s:e]]                          # [slab, 2*TOPB, F]
        dd = (g2 * d1T[s:e, None, :]).sum(-1)        # [slab, 2*TOPB] f32
        m1 = dd.max(axis=1)
        nn = np.where(dd == m1[:, None], rows[s:e], N_KPTS + 1).min(axis=1)
        mk = np.where(rows[s:e] == nn[:, None], -np.inf, dd)
        cm1[s:e] = m1
        cm2[s:e] = mk.max(axis=1)
        bck_nn[s:e] = nn

    # ---- exact reference arithmetic (float32) ----
    def dist(s):
        return SQRT_2 * np.sqrt(np.maximum(ONE - s.astype(np.float32),
                                           CLIP_LO))

    fwd_ok = (dist(s1) / dist(s2)) < ONE
    bck_ok = (dist(cm1) / dist(cm2)) < ONE

    mutual = fwd_ok & bck_ok[fwd_nn] & (bck_nn[fwd_nn] == np.arange(N_KPTS))

    indices0 = np.where(mutual, fwd_nn, -1)[None, :].astype(np.int32)
    mscores0 = (indices0 > 0).astype(np.int32)
    matches1 = np.full((1, M_KPTS), -1, dtype=np.int32)
    mscores1 = np.zeros((1, M_KPTS), dtype=np.float32)
    return indices0, matches1, mscores0, mscores1


def kernel(descriptors0, descriptors1, keypoints0, keypoints1):
    d0 = np.ascontiguousarray(descriptors0[0]).astype(np.float32, copy=False)
    d1 = np.ascontiguousarray(descriptors1[0]).astype(np.float32, copy=False)
    results = run_device(d0, d1)
    return postprocess(results, d0, d1)
